# revision 18
# baseline (speedup 1.0000x reference)
"""Multi-head attention (B=4, S=2048, E=1024, H=16, hd=64) on 8 TRN2 cores.

Sharding: core c -> batch b = c//2, head-half hh = c%2 (8 heads = 512 internal
dims).  Data parallel on B, tensor parallel on heads.  Each core computes a
partial out-projection for its batch; the host sums the two head-half partials
per batch and adds the (folded) output bias.

Device dataflow (bf16 matmuls, fp32 PSUM accumulation):
  - host pre-transposes q/k/v to (E, S), pre-permutes the weights into the
    [128, ktile, n] SBUF layout (contiguous weight DMAs, no strided gathers).
  - DMA order is the critical path: wk/wq, all of kT, the first two quarters
    of qT, then vT in column chunks, wo late.  Attention starts once khT m0
    and the first qhT quarter exist; the v-projection interleaves into the
    first query chunk as filler work while vT chunks stream in.  qT quarters
    2,3 are DMAed into the same half-size stage tiles after the quarter-0/1
    projection chains have consumed them (quarter-major q projection).
  - softmax exponentials computed as 2^t (log2(e)/8 folded into qhT): even
    key-tiles on ScalarE (activation Exp with scale=ln2), odd key-tiles on
    the vector engine via the Schraudolph bit trick -- int16(128*t + 16250.5)
    reinterpreted as bf16 IS 2^t to ~1% -- so consecutive exps run on two
    engines concurrently: the 2-buffer PSUM score pool (each score tile is
    freed by its exp) turns over at the pair rate and attention stays
    TensorE-paced.
  - attention emitted in kt pairs ([scores k0, scores k1] [fillers]
    [AV k0, AV k1]) so the PE array switches between the K=64 scores config
    and the K=128 AV/projection config half as often.
  - AV matmuls: lhsT = vh[:, h*65 : h*65+65] (64 value dims + ones column
    accumulating the softmax denominator in PSUM row 64).
  - division: AV evacuated PSUM->SBUF on ScalarE (frees the accumulator),
    denominator row DMA round-trips through DRAM to broadcast across
    partitions; the DVE reciprocal+multiply chain is deferred to the middle
    of the NEXT query chunk so its DMA wait never blocks the odd-kt exps
    sharing the DVE queue.
"""

import math
import sys
from contextlib import ExitStack

sys.path.insert(0, "/opt/trn_rl_repo")

import numpy as np
import ml_dtypes

import concourse.bass as bass
from concourse import bacc
import concourse.mybir as mybir
import concourse.tile as tile

F32 = mybir.dt.float32
BF16 = mybir.dt.bfloat16
I16 = mybir.dt.int16
AF = mybir.ActivationFunctionType
ALU = mybir.AluOpType

B, S, E = 4, 2048, 1024
H, HD = 16, 64
HLOC = 8          # heads per core
ILOC = HLOC * HD  # 512 internal dims per core
KT = E // 128     # 8 embed k-tiles
ST = S // 128     # 16 seq tiles
NCORES = 8
SCALE = 1.0 / math.sqrt(HD)  # 1/8
LOG2E = math.log2(math.e)
LN2 = math.log(2.0)
# odd key-tiles exp on the vector engine (Schraudolph): consecutive kt exps
# run on different engines and overlap
DVE_KT = tuple(range(1, ST, 2))
VROW = HLOC * 65  # 520: 8 x (64 value dims + ones column)


def build_nc():
    nc = bacc.Bacc()

    qT_d = nc.declare_dram_parameter("qT", [E, S], BF16, isOutput=False).ap()
    kT_d = nc.declare_dram_parameter("kT", [E, S], BF16, isOutput=False).ap()
    vT_d = nc.declare_dram_parameter("vT", [E, S], BF16, isOutput=False).ap()
    wq_d = nc.declare_dram_parameter("wq", [128, KT, ILOC], BF16,
                                     isOutput=False).ap()
    wk_d = nc.declare_dram_parameter("wk", [128, KT, ILOC], BF16,
                                     isOutput=False).ap()
    wv_d = nc.declare_dram_parameter("wv", [128, KT, ILOC], BF16,
                                     isOutput=False).ap()
    wo_d = nc.declare_dram_parameter("wo", [128, 4, E], BF16,
                                     isOutput=False).ap()
    bq_d = nc.declare_dram_parameter("bq", [128, 4], F32, isOutput=False).ap()
    bk_d = nc.declare_dram_parameter("bk", [128, 4], F32, isOutput=False).ap()
    out_d = nc.declare_dram_parameter("out", [S, E], F32, isOutput=True).ap()
    dscr = nc.dram_tensor("dscratch", [32, 512], F32).ap()

    with tile.TileContext(nc) as tc, ExitStack() as ctx:
        # ---- pools (PSUM: pp 2x1 + sc 2x2 + av 2x1 = 8 banks) ----
        psum = ctx.enter_context(tc.tile_pool(name="psum", bufs=2, space="PSUM"))
        scp = ctx.enter_context(tc.tile_pool(name="scp", bufs=4, space="PSUM"))
        av_pool = ctx.enter_context(tc.tile_pool(name="avp", bufs=2, space="PSUM"))
        qhT_pool = ctx.enter_context(tc.tile_pool(name="qhT", bufs=4))
        khT_pool = ctx.enter_context(tc.tile_pool(name="khT", bufs=2))
        vh_pool = ctx.enter_context(tc.tile_pool(name="vh", bufs=ST))
        bias_pool = ctx.enter_context(tc.tile_pool(name="bias", bufs=1))
        wpool = ctx.enter_context(tc.tile_pool(name="w_in", bufs=3))
        wo_pool = ctx.enter_context(tc.tile_pool(name="w_o", bufs=1))
        stgk_pool = ctx.enter_context(tc.tile_pool(name="stgk", bufs=KT))
        stgq_pool = ctx.enter_context(tc.tile_pool(name="stgq", bufs=KT))
        stgv_pool = ctx.enter_context(tc.tile_pool(name="stgv", bufs=KT))
        exp_pool = ctx.enter_context(tc.tile_pool(name="exp", bufs=6))
        attnT_pool = ctx.enter_context(tc.tile_pool(name="attnT", bufs=4))
        small_pool = ctx.enter_context(tc.tile_pool(name="small", bufs=2))
        bc_pool = ctx.enter_context(tc.tile_pool(name="bcb", bufs=2))
        tmp_pool = ctx.enter_context(tc.tile_pool(name="tmpp", bufs=1))
        out_pool = ctx.enter_context(tc.tile_pool(name="outbuf", bufs=2))

        vh = [vh_pool.tile([128, VROW], BF16, tag="vh", name=f"vh{i}")
              for i in range(ST)]
        attnT = [attnT_pool.tile([128, S], BF16, tag="attnT",
                                 name=f"attnT{i}") for i in range(4)]
        # khT/qhT rings: tiles allocated lazily when a projection chain first
        # writes them (khT pool holds only 2 generations)
        kh_tiles = {}
        qh_tiles = {}

        def get_dst(which, m):
            tiles, pool = (kh_tiles, khT_pool) if which == "k" \
                else (qh_tiles, qhT_pool)
            if m not in tiles:
                tiles[m] = pool.tile([128, S], BF16, tag=which + "hT",
                                     name=f"{which}hT{m}")
            return tiles[m]

        bq_t = bias_pool.tile([128, 4], F32, tag="bq")
        bk_t = bias_pool.tile([128, 4], F32, tag="bk")
        nc.sync.dma_start(bq_t[:], bq_d[:])
        nc.sync.dma_start(bk_t[:], bk_d[:])

        wq_t = wpool.tile([128, KT, ILOC], BF16, tag="w", name="wq")
        wk_t = wpool.tile([128, KT, ILOC], BF16, tag="w", name="wk")
        wv_t = wpool.tile([128, KT, ILOC], BF16, tag="w", name="wv")
        wo_holder = {}

        # ones columns for the denominator trick
        for st in range(ST):
            ones = vh[st][:].rearrange("p (h x) -> p h x", x=65)[:, :, 64:65]
            nc.vector.memset(ones, 1.0)

        # ---- staged loads in critical-path order ----
        nc.sync.dma_start(wk_t[:], wk_d[:])
        nc.sync.dma_start(wq_t[:], wq_d[:])
        stg_k = [stgk_pool.tile([128, S], BF16, tag="stage", name=f"stgk{kk}")
                 for kk in range(KT)]
        for kk in range(KT):
            nc.sync.dma_start(stg_k[kk][:], kT_d[kk * 128:(kk + 1) * 128, :])
        # q stage: half-size tiles; quarters 2,3 later overwrite quarters 0,1
        stg_q = [stgq_pool.tile([128, 1024], BF16, tag="stage",
                                name=f"stgq{kk}") for kk in range(KT)]
        for qq in range(2):
            for kk in range(KT):
                nc.sync.dma_start(
                    stg_q[kk][:, qq * 512:(qq + 1) * 512],
                    qT_d[kk * 128:(kk + 1) * 128, qq * 512:(qq + 1) * 512])
        nc.sync.dma_start(wv_t[:], wv_d[:])
        stg_v = [stgv_pool.tile([128, S], BF16, tag="stage", name=f"stgv{kk}")
                 for kk in range(KT)]
        for vc in range(4):
            cols = slice(vc * 512, (vc + 1) * 512)
            for kk in range(KT):
                nc.sync.dma_start(stg_v[kk][:, cols],
                                  vT_d[kk * 128:(kk + 1) * 128, cols])

        def vproj_ops():
            """Closure list projecting vh[st] chains (fillers for chunk 0)."""
            ops = []
            for st in range(ST):
                holder = {}
                for kk in range(KT):
                    def mm(st=st, kk=kk, holder=holder, first=(kk == 0)):
                        if first:
                            holder["ps"] = psum.tile(
                                [128, 512], F32, tag="pp", name="psv")
                        nc.tensor.matmul(
                            holder["ps"][:],
                            lhsT=stg_v[kk][:, st * 128:(st + 1) * 128],
                            rhs=wv_t[:, kk, :],
                            start=(kk == 0), stop=(kk == KT - 1),
                        )
                    ops.append(mm)

                def evac(st=st, holder=holder):
                    pin = holder["ps"][:].rearrange("p (h x) -> p h x", x=64)
                    pout = vh[st][:].rearrange(
                        "p (h c) -> p h c", c=65)[:, :, 0:64]
                    nc.vector.tensor_copy(pout, pin)
                ops.append(evac)
            return ops

        def proj_chain(which, m, quarter):
            """Closure list for one khT/qhT m-tile x column-quarter chain."""
            if which == "k":
                stg, w_t, b_t, scale = stg_k, wk_t, bk_t, 1.0
                scol = quarter * 512
            else:
                stg, w_t, b_t, scale = stg_q, wq_t, bq_t, SCALE * LOG2E
                scol = (quarter % 2) * 512
            cols = slice(quarter * 512, (quarter + 1) * 512)
            holder = {}
            ops = []

            for kk in range(KT):
                def mm(kk=kk, holder=holder, first=(kk == 0)):
                    if first:
                        holder["dst"] = get_dst(which, m)
                        holder["ps"] = psum.tile(
                            [128, 512], F32, tag="pp", name="psqk")
                    nc.tensor.matmul(
                        holder["ps"][:],
                        lhsT=w_t[:, kk, m * 128:(m + 1) * 128],
                        rhs=stg[kk][:, scol:scol + 512],
                        start=(kk == 0), stop=(kk == KT - 1),
                    )
                ops.append(mm)

            def evac(holder=holder):
                nc.scalar.activation(
                    holder["dst"][:, cols], holder["ps"][:], AF.Identity,
                    bias=b_t[:, m:m + 1], scale=scale,
                )
            ops.append(evac)
            return ops

        def q23_dma_op():
            def op():
                for qq in range(2, 4):
                    for kk in range(KT):
                        nc.sync.dma_start(
                            stg_q[kk][:, (qq % 2) * 512:(qq % 2) * 512 + 512],
                            qT_d[kk * 128:(kk + 1) * 128,
                                 qq * 512:(qq + 1) * 512])
            return [op]

        def wo_dma_op():
            def op():
                wo_holder["t"] = wo_pool.tile([128, 4, E], BF16, tag="wo",
                                              name="wo_t")
                nc.sync.dma_start(wo_holder["t"][:], wo_d[:])
            return [op]

        def outproj_ops(qc):
            """Closure list projecting output for query chunk qc."""
            ops = []
            for qt in range(qc * 4, qc * 4 + 4):
                holder = {}

                for c in range(2):
                    for it in range(4):
                        def mm(qt=qt, c=c, it=it, holder=holder,
                               first=(c == 0 and it == 0)):
                            if first:
                                holder["ot"] = out_pool.tile(
                                    [128, 1024], F32, tag="ot", name="ot")
                            if it == 0:
                                holder["po"] = psum.tile(
                                    [128, 512], F32, tag="pp", name="po")
                            nc.tensor.matmul(
                                holder["po"][:],
                                lhsT=attnT[it][:, qt * 128:(qt + 1) * 128],
                                rhs=wo_holder["t"][:, it,
                                                   c * 512:(c + 1) * 512],
                                start=(it == 0), stop=(it == 3),
                            )
                        ops.append(mm)

                    def evac(qt=qt, c=c, holder=holder, last=(c == 1)):
                        nc.scalar.activation(
                            holder["ot"][:, c * 512:(c + 1) * 512],
                            holder["po"][:], AF.Copy)
                        if last:
                            nc.sync.dma_start(
                                out_d[qt * 128:(qt + 1) * 128, :],
                                holder["ot"][:])
                    ops.append(evac)
            return ops

        def division_front(g, qc, avA, avB):
            """Evacuate AV + launch the denominator DMA round-trip."""
            idx = (g * 4 + qc) * 2
            avsA = small_pool.tile([65, 512], F32, tag="avs", name="avsA")
            avsB = small_pool.tile([65, 512], F32, tag="avs", name="avsB")
            nc.scalar.activation(avsA[:], avA[:], AF.Copy)
            nc.scalar.activation(avsB[:], avB[:], AF.Copy)
            nc.sync.dma_start(dscr[idx:idx + 1, :], avsA[64:65, :])
            nc.sync.dma_start(dscr[idx + 1:idx + 2, :], avsB[64:65, :])
            bcA = bc_pool.tile([64, 512], F32, tag="bc", name="bcA")
            bcB = bc_pool.tile([64, 512], F32, tag="bc", name="bcB")
            nc.sync.dma_start(
                bcA[:].rearrange("p (o n) -> p o n", o=1),
                dscr[idx, :].partition_broadcast(64))
            nc.sync.dma_start(
                bcB[:].rearrange("p (o n) -> p o n", o=1),
                dscr[idx + 1, :].partition_broadcast(64))
            return (g, qc, avsA, avsB, bcA, bcB)

        def division_back(pend):
            """DVE reciprocal+multiply, deferred so the DMA wait is over."""
            g, qc, avsA, avsB, bcA, bcB = pend
            qcols = slice(qc * 512, (qc + 1) * 512)
            nc.vector.reciprocal_approx_fast(bcA[:], bcA[:])
            nc.vector.reciprocal_approx_fast(bcB[:], bcB[:])
            nc.vector.tensor_mul(attnT[g][0:64, qcols], avsA[0:64, :], bcA[:])
            tmp = tmp_pool.tile([64, 512], BF16, tag="tmp", name="tmp")
            nc.vector.tensor_mul(tmp[:], avsB[0:64, :], bcB[:])
            nc.sync.dma_start(attnT[g][64:128, qcols], tmp[:])

        # upfront: khT m0 fully + qhT m0 quarter 0; the rest interleaves
        for quarter in range(4):
            for op in proj_chain("k", 0, quarter):
                op()
        for op in proj_chain("q", 0, 0):
            op()

        pending_div = []

        for g in range(4):              # head pair (2g, 2g+1)
            hA, hB = 2 * g, 2 * g + 1
            if g == 0:
                head_fillers = vproj_ops()   # drained during qc 0
                fillers = []
                # quarter-major q chains free the q stage halves for the
                # quarter-2/3 DMA; khT[1] afterwards (needed at g=1)
                for mq in ((0, 1), (1, 0), (1, 1), (2, 0), (2, 1),
                           (3, 0), (3, 1)):
                    fillers += proj_chain("q", *mq)
                fillers += q23_dma_op()
                for mq in ((0, 2), (0, 3), (1, 2), (1, 3)):
                    fillers += proj_chain("q", *mq)
                for quarter in range(4):
                    fillers += proj_chain("k", 1, quarter)
                for mq in ((2, 2), (2, 3), (3, 2), (3, 3)):
                    fillers += proj_chain("q", *mq)
            elif g < 3:
                head_fillers = []
                fillers = []
                for quarter in range(4):
                    fillers += proj_chain("k", g + 1, quarter)
                if g == 2:
                    fillers = wo_dma_op() + fillers
            else:
                head_fillers = []
                fillers = []
            steps_left = 4 * ST
            pending_outproj = []
            for qc in range(4):         # 512-query chunks
                if g == 3 and qc >= 1:
                    # must wait for division_back(qc-1) before entering the
                    # filler stream: out-projection reads attnT[3][qc-1]
                    pending_outproj = outproj_ops(qc - 1)
                qcols = slice(qc * 512, (qc + 1) * 512)
                avA = av_pool.tile([65, 512], F32, tag="av", name="avA")
                avB = av_pool.tile([65, 512], F32, tag="av", name="avB")
                for kp in range(ST // 2):
                    pair = (2 * kp, 2 * kp + 1)
                    # v-projection fillers: 2x9/step emits vh[kt]'s chain
                    # before the AV matmul that consumes it
                    for _ in range(min(18, len(head_fillers))):
                        head_fillers.pop(0)()
                    exs = {}
                    for kt in pair:
                        scA = scp.tile([128, 512], F32, tag="sc", name="scA")
                        scB = scp.tile([128, 512], F32, tag="sc", name="scB")
                        nc.tensor.matmul(
                            scA[:],
                            lhsT=kh_tiles[g][0:64, kt * 128:(kt + 1) * 128],
                            rhs=qh_tiles[g][0:64, qcols],
                            start=True, stop=True,
                        )
                        nc.tensor.matmul(
                            scB[:],
                            lhsT=kh_tiles[g][64:128, kt * 128:(kt + 1) * 128],
                            rhs=qh_tiles[g][64:128, qcols],
                            start=True, stop=True,
                        )
                        # each kt: one head's exp exact on ScalarE, the other
                        # Schraudolph on DVE -- the two run concurrently and
                        # the 1-bank sc buffers turn over at half-exp latency
                        exA = exp_pool.tile([128, 512], BF16, tag="exp",
                                            name="exA")
                        exB = exp_pool.tile([128, 512], BF16, tag="exp",
                                            name="exB")
                        if kt % 2 == 0:
                            sc_s, ex_s, sc_v, ex_v = scA, exA, scB, exB
                        else:
                            sc_s, ex_s, sc_v, ex_v = scB, exB, scA, exA
                        nc.scalar.activation(ex_s[:], sc_s[:], AF.Exp,
                                             scale=LN2)
                        # Schraudolph 2^t: bf16 = i16(128*t + 16250.5)
                        nc.vector.tensor_scalar(
                            out=ex_v[:].bitcast(I16), in0=sc_v[:],
                            scalar1=128.0, scalar2=16250.5,
                            op0=ALU.mult, op1=ALU.add,
                        )
                        exs[kt] = (exA, exB)
                    # deferred division tail from the previous chunk
                    if kp == 4 and pending_div:
                        division_back(pending_div.pop(0))
                        if pending_outproj:
                            fillers.extend(pending_outproj)
                            pending_outproj = []
                    # pace interleaved filler work (proj / out-proj)
                    steps_left -= 2
                    n_take = -(-(2 * len(fillers)) // max(steps_left, 2)) \
                        if fillers else 0
                    for _ in range(min(n_take, len(fillers))):
                        fillers.pop(0)()
                    for kt in pair:
                        first, last = (kt == 0), (kt == ST - 1)
                        nc.tensor.matmul(
                            avA[:],
                            lhsT=vh[kt][:, hA * 65:hA * 65 + 65],
                            rhs=exs[kt][0][:],
                            start=first, stop=last,
                        )
                        nc.tensor.matmul(
                            avB[:],
                            lhsT=vh[kt][:, hB * 65:hB * 65 + 65],
                            rhs=exs[kt][1][:],
                            start=first, stop=last,
                        )
                pending_div.append(division_front(g, qc, avA, avB))
            # flush any leftover fillers for this pair
            for op in fillers:
                op()

        # drain: last division + final out-projection chunk
        while pending_div:
            division_back(pending_div.pop(0))
        for op in outproj_ops(3):
            op()

    nc.finalize()
    return nc


def make_in_maps(q, k, v, Wq, bq, Wk, bk, Wv, bv, Wo, bo):
    """Per-core input dicts + the folded host-side bias."""
    bf = ml_dtypes.bfloat16

    def wprep(w, n):  # [E or ILOC, n] -> [128, ktiles, n] contiguous
        kt = w.shape[0] // 128
        return np.ascontiguousarray(
            w.reshape(kt, 128, n).transpose(1, 0, 2)).astype(bf)

    qT = [np.ascontiguousarray(q[b].T).astype(bf) for b in range(B)]
    kT = [np.ascontiguousarray(k[b].T).astype(bf) for b in range(B)]
    vT = [np.ascontiguousarray(v[b].T).astype(bf) for b in range(B)]
    in_maps = []
    for c in range(NCORES):
        b, hh = divmod(c, 2)
        isl = slice(hh * ILOC, (hh + 1) * ILOC)
        bq_loc = np.ascontiguousarray(
            (bq[isl] * SCALE * LOG2E).reshape(4, 128).T)
        bk_loc = np.ascontiguousarray(bk[isl].reshape(4, 128).T)
        in_maps.append({
            "qT": qT[b], "kT": kT[b], "vT": vT[b],
            "wq": wprep(Wq[:, isl], ILOC),
            "wk": wprep(Wk[:, isl], ILOC),
            "wv": wprep(Wv[:, isl], ILOC),
            "wo": wprep(Wo[isl, :], E),
            "bq": bq_loc, "bk": bk_loc,
        })
    bo_eff = (bo + bv @ Wo).astype(np.float32)
    return in_maps, bo_eff


_NC_CACHE = None


def kernel(q, k, v, Wq, bq, Wk, bk, Wv, bv, Wo, bo):
    global _NC_CACHE
    from concourse.bass_utils import run_bass_kernel_spmd

    if _NC_CACHE is None:
        _NC_CACHE = build_nc()
    nc = _NC_CACHE
    in_maps, bo_eff = make_in_maps(q, k, v, Wq, bq, Wk, bk, Wv, bv, Wo, bo)
    res = run_bass_kernel_spmd(nc, in_maps, list(range(NCORES)))
    out = np.empty((B, S, E), np.float32)
    for b in range(B):
        out[b] = res.results[2 * b]["out"] + res.results[2 * b + 1]["out"] + bo_eff
    return out
